# revision 55
# baseline (speedup 1.0000x reference)
"""MoE layer (top-2 of 8 experts, exact GELU) on 8 Trainium2 NeuronCores.

Strategy: expert parallelism. The router (0.006% of the FLOPs) runs on the
host; each core holds one expert's weights and runs the two big matmuls
for the tokens routed to that expert:

    H^T = W1[e]^T @ X_e^T          (PE, bf16, accumulated over C in fp32)
    G   = GELU(H^T + b1)           (ACT, fused bias, bf16 out)
    Y   = G^T @ W2[e]              (PE, bf16, accumulated over D in fp32)

Schedule: the kernel is DMA-limited at the start (19 MB of weights +
activations to land) and PE-limited after. Phase 1 runs ALL of matmul1,
d-group-outer, streaming w1 through a 3-deep tile pool -- the PE only
needs xt (2.2 MB) + one 1 MB w1 group to start, and consumes w1 at
~65 GB/s while w2 (8.4 MB) streams in the background. Phase 2 runs all
of matmul2 from the then-fully-resident w2. G for all tokens stays in
SBUF between the phases (freed w1 budget pays for it). This keeps the
PE saturated from ~10us on instead of stalling ~45us for weight groups
(which also dropped the HAM clock gate to half rate).

Computing H transposed (d on partitions) makes the first matmul's output
directly usable as the second matmul's stationary operand -- no on-device
transposes anywhere.
"""

import numpy as np
import ml_dtypes

B, T, C, D, E = 2, 2048, 1024, 4096, 8
N = B * T
TOP_K = 2
NT_BLOCK = 384          # token block width (psum tile free dim)
WARMUP = 30             # PE warm-up matmuls (HAM clock ramp + DMA wait)

_cache = {}


MAX_WAITS = 1  # this walrus build rejects >1 sync wait per instruction


def _install_tile_patch():
    """This container's walrus build rejects >MAX_WAITS sync waits on one
    instruction ("Too many sync wait commands"). Two fixes:
    1. The stock TileContext tail puts every outstanding proc-clock wait on
       a single Drain -- split across sync-engine NOPs, one wait each.
    2. Body instructions can come out of sem assignment with 3+ waits --
       peel the excess onto same-engine NOPs inserted just before."""
    import bass_rust
    import concourse.mybir as mybir
    from concourse import tile
    from concourse.vector_clock import ScopedClock

    if getattr(tile.TileContext, "_drain_patched", False):
        return

    def _patched(self, tick_clock, wait_clock):
        nc = self.nc
        ticks = list(tick_clock.global_clock)
        # round-robin the tail waits across engines: serial on one
        # engine they cost ~1.4us before the final barrier
        engs = [nc.sync, nc.scalar, nc.vector, nc.gpsimd, nc.tensor]
        n = 0
        for p, t in enumerate(ticks):
            if t <= 0:
                continue
            vc = bass_rust.VectorClock()
            vc.require_at_least(p, t)
            nop = engs[n % len(engs)].nop(nofuse=True, hint=f"tail_wait_p{p}")
            n += 1
            wait_clock.add_sem_waits(nop.ins, ScopedClock({None: vc}))
        nc.sync.drain()
        nc.all_engine_barrier()
        assert self.sems is not None
        popped = nc._tile_sem_poison_stack.pop()
        assert popped is self._sem_poison
        nc.clear_and_free_semaphores(list(self.sems.allocated().values()))
        nc.all_engine_barrier()

    tile.TileContext._drain_and_barrier = _patched

    orig_lower = tile.TileContext._lower_ordered_insts

    def _split_excess_waits(self, ordered):
        nc = self.nc
        for bb_name, insts in ordered.items():
            out = []
            for inst in insts:
                si = getattr(inst, "sync_info", None)
                if (
                    si is not None
                    and si.on_wait
                    and len(si.on_wait) > MAX_WAITS
                    and isinstance(inst, mybir.Instruction)
                    and inst.engine != mybir.EngineType.Unassigned
                ):
                    waits = list(si.on_wait)
                    excess, keep = waits[:-MAX_WAITS], waits[-MAX_WAITS:]
                    while excess:
                        chunk, excess = excess[:MAX_WAITS], excess[MAX_WAITS:]
                        nop = mybir.InstNoOp(
                            name=nc.get_next_instruction_name(),
                            sync_info=mybir.SyncInfo(on_wait=chunk, on_update=[]),
                            bass_nofuse=True,
                            engine=inst.engine,
                        )
                        nc.register_instruction(nop)
                        out.append(nop)
                    inst.sync_info = mybir.SyncInfo(
                        on_wait=keep, on_update=list(si.on_update or [])
                    )
                out.append(inst)
            insts[:] = out
        return orig_lower(self, ordered)

    tile.TileContext._lower_ordered_insts = _split_excess_waits
    tile.TileContext._drain_patched = True


def _blocks_of(cap):
    """Token-block widths covering cap. Full NT_BLOCK blocks plus one
    remainder block (kept last so the writeback tail is small)."""
    blocks = [NT_BLOCK] * (cap // NT_BLOCK)
    if cap % NT_BLOCK:
        blocks.append(cap % NT_BLOCK)
    return blocks


def _build(cap):
    """Build the per-core Bass program for token capacity `cap`
    (a multiple of 8)."""
    import concourse.bass as bass
    import concourse.mybir as mybir
    import concourse.tile as tile
    from contextlib import ExitStack

    _install_tile_patch()

    bf16 = mybir.dt.bfloat16
    f32 = mybir.dt.float32
    KC = C // 128            # 8 contraction chunks for matmul1
    DT = D // 128            # 32 d-tiles / contraction chunks for matmul2
    WG = 512                 # w1 d-columns per group (4 d-tiles)
    NG = D // WG             # 8 groups
    NQ = DT // 4             # 8 w2 chunks
    NB = C // 128            # 8 output-channel tiles for matmul2
    GDT = WG // 128          # 4 d-tiles per w1 group
    blocks = _blocks_of(cap)
    t0s = [sum(blocks[:i]) for i in range(len(blocks))]

    nc = bass.Bass()
    # Inputs are host-pre-tiled so every DMA lands with a large contiguous
    # per-partition run (>=6 KB): strided destinations chop transfers into
    # ~1 KB packets that run an HWDGE ring at a fraction of peak.
    #   xt : [128, KC, cap]    xt[p, kc, t]  = x^T[kc*128+p, t]  (block-packed)
    #   w1 : [NG, 128, KC, WG] w1[g, p, kc, j] = w1[kc*128+p, g*WG+j]
    #   w2 : [NQ, 128, 4, C]   w2[q, p, a, c] = w2[(4q+a)*128+p, c]
    # xt is packed block-contiguous: block b at columns [KC*t0, KC*(t0+bw))
    # as (kc, t) so each per-block DMA is one contiguous run per partition.
    xt = nc.declare_dram_parameter("xt", [128, KC * cap], bf16, isOutput=False)
    w1 = nc.declare_dram_parameter("w1", [NG, 128, KC, WG], bf16, isOutput=False)
    w2 = nc.declare_dram_parameter("w2", [NQ, 128, 4, C], bf16, isOutput=False)
    b1t = nc.declare_dram_parameter("b1t", [128, DT], f32, isOutput=False)
    # y^T, block-packed cb-major: block b at columns [NB*t0, NB*(t0+bw))
    # as (cb, t); y[p, NB*t0 + cb*bw + t] = y^T[cb*128+p, t0+t]
    y = nc.declare_dram_parameter("y", [128, NB * cap], bf16, isOutput=True)

    from concourse.bass import _add_dep_helper

    with tile.TileContext(nc) as tc, ExitStack() as ctx:
        const = ctx.enter_context(tc.tile_pool(name="const", bufs=1))
        # per-block xt tiles: each DMA fills a whole tile, so the
        # destination is one contiguous ~6 KB run per partition (a column
        # slice of a shared [128, KC, cap] tile would chop the transfer
        # into 768 B packets and crawl)
        xts = [const.tile([128, KC, bw], bf16, name=f"xts{i}")
               for i, bw in enumerate(blocks)]
        w2s = const.tile([128, DT, C], bf16)
        g = const.tile([128, DT, cap], bf16)
        b1s = const.tile([128, DT], f32)

        w1p = ctx.enter_context(tc.tile_pool(name="w1p", bufs=4))
        ps1 = ctx.enter_context(tc.tile_pool(name="ps1", bufs=4, space="PSUM"))
        ps2 = ctx.enter_context(tc.tile_pool(name="ps2", bufs=3, space="PSUM"))
        yev = ctx.enter_context(tc.tile_pool(name="yev", bufs=2))
        warm = ctx.enter_context(tc.tile_pool(name="warm", bufs=1))

        # DMA plan. Per-ring order == consumption order:
        #   scalar ring: b1, then w1 groups (the matmul1 critical path --
        #       nothing else ever rides this ring, so groups can't be
        #       crowded out; g3..g7 are kicked from inside the loop as
        #       the 3-deep pool recycles).
        #   sync ring: xt blocks (needed within the first w1-group pass),
        #       then half of w2 behind them.
        #   gpsimd SWDGE ring (~13us spin-up): the other half of w2,
        #       which matmul2 only touches after all of matmul1.
        def xt_src(blk):
            t0, bw = t0s[blk], blocks[blk]
            return xt[:, KC * t0 : KC * (t0 + bw)].rearrange(
                "p (kc t) -> p kc t", kc=KC)

        # Ring plan. A single HWDGE ring sustains only ~80-130 GB/s, and
        # the sync ring's first packet lands ~2us before the scalar
        # ring's -- so the w1 g0 halves (stationary for the first psum
        # groups) lead the EARLY sync ring and xt b0 leads scalar. The
        # first block's psum groups open on kc 0-3 (needs only the h1
        # pieces) and close on kc 4-7 as the h2 pieces land:
        #   sync:   w1 g0 (2 kc-halves), xt b2, then w2 odd chunks (gated)
        #   scalar: b1, xt b0 (2 kc-halves), w1 g1-g3 (g4-g7 in-loop)
        #   gpsimd: xt b1, then w2 even chunks (gated)
        w1t = []
        for grp in range(4):
            w1t.append(w1p.tile([128, KC, WG], bf16, tag="w1",
                                name=f"w1t{grp}"))
        nc.sync.dma_start(w1t[0][:, 0:4, :], w1[0][:, 0:4, :])
        nc.scalar.dma_start(b1s[:], b1t[:])
        nc.gpsimd.dma_start(xts[1][:], xt_src(1))
        nc.scalar.dma_start(xts[0][:, 0:4, :], xt_src(0)[:, 0:4, :])
        nc.sync.dma_start(w1t[0][:, 4:8, :], w1[0][:, 4:8, :])
        nc.scalar.dma_start(xts[0][:, 4:8, :], xt_src(0)[:, 4:8, :])
        nc.sync.dma_start(xts[2][:], xt_src(2))
        for grp in range(1, 4):
            nc.scalar.dma_start(w1t[grp][:], w1[grp])
        # w2 is only touched in phase 2 (~125us in); hold its 8.4 MB off
        # the HBM bus until the matmul1 critical path (xt + early w1
        # groups) has landed. A raw dep edge to the w1 DMA only orders
        # the descriptor KICKS, not the transfers, and the Tile scheduler
        # reorders an engine's kicks past a blocked one -- so EVERY chunk
        # gets a real data hazard: a dummy copy that reads a late w1 tile
        # and writes a corner of the w2 destination, which the w2 DMA
        # must wait out (WAW).
        for q in range(NQ):
            eng = nc.gpsimd if q % 2 == 0 else nc.sync
            nc.vector.tensor_copy(w2s[:, 4 * q : 4 * q + 1, 0:1],
                                  w1t[2 + (q % 2)][:, 0:1, 0:1])
            eng.dma_start(w2s[:, 4 * q : 4 * (q + 1), :], w2[q])

        # PE warm-up: the HAM clock gate needs ~3.4us of sustained matmul
        # activity to lift the PE from 1.2 to 2.4 GHz, and the first real
        # matmul can't start until xt block 0 + w1 group 0 land (~12us).
        # Burn the wait on dummy matmuls over zeroed scratch, ping-ponging
        # two psum tiles borrowed from the (idle) ps2 pool -- a single
        # tile WAW-serializes each matmul on the previous one's drain.
        wsrc = warm.tile([128, NT_BLOCK], bf16)
        nc.vector.memset(wsrc[:], 0.0)
        wps = [ps2.tile([128, NT_BLOCK], f32, tag="py", name=f"wps{i}")
               for i in range(2)]
        for i in range(WARMUP):
            nc.tensor.matmul(wps[i % 2][:], wsrc[:, :128], wsrc[:],
                             start=True, stop=True)

        # Phase 1: all of matmul1, d-group-outer so each 1 MB w1 group is
        # fully consumed (4 d-tiles x all token blocks) before the next is
        # needed -- ~15us of PE work per group vs ~4us to stream one in.
        # Block-outer within the group so the very first psum group needs
        # only xt b0 (not all three blocks).
        for grp in range(NG):
            for blk, bw in enumerate(blocks):
                t0 = t0s[blk]
                if grp == 0 and blk == 0:
                    # First block: all four psum groups open on kc 0-3,
                    # close on kc 4-7, so the PE starts as soon as the
                    # h1 DMA pieces land instead of waiting for all of
                    # w1 g0 + xt b0.
                    phs = [ps1.tile([128, bw], f32, tag="ph",
                                    name=f"ph0_{dtl}") for dtl in range(GDT)]
                    for half in range(2):
                        for dtl in range(GDT):
                            for kc in range(4 * half, 4 * half + 4):
                                nc.tensor.matmul(
                                    phs[dtl][:],
                                    w1t[0][:, kc, 128 * dtl : 128 * (dtl + 1)],
                                    xts[0][:, kc, :],
                                    start=(kc == 0),
                                    stop=(kc == KC - 1),
                                )
                    for dtl in range(GDT):
                        nc.scalar.activation(
                            g[:, dtl, t0 : t0 + bw], phs[dtl][:],
                            mybir.ActivationFunctionType.Gelu,
                            bias=b1s[:, dtl : dtl + 1],
                        )
                    continue
                for dtl in range(GDT):
                    dt = GDT * grp + dtl
                    ph = ps1.tile([128, bw], f32, tag="ph")
                    for kc in range(KC):
                        nc.tensor.matmul(
                            ph[:],
                            w1t[grp][:, kc, 128 * dtl : 128 * (dtl + 1)],
                            xts[blk][:, kc, :],
                            start=(kc == 0),
                            stop=(kc == KC - 1),
                        )
                    nc.scalar.activation(
                        g[:, dt, t0 : t0 + bw], ph[:],
                        mybir.ActivationFunctionType.Gelu,
                        bias=b1s[:, dt : dt + 1],
                    )
            if grp + 4 < NG:
                w1t.append(w1p.tile([128, KC, WG], bf16, tag="w1",
                                    name=f"w1t{grp + 4}"))
                nc.scalar.dma_start(w1t[grp + 4][:], w1[grp + 4])

        # Phase 2: all of matmul2 from resident w2, cb-outer per block.
        # y^T: lhsT = w2 tile (stationary), rhs = g (tokens moving). The
        # last (smallest) block writes back per-cb so only a sliver of
        # DMA trails the final matmul.
        for blk, bw in enumerate(blocks):
            t0 = t0s[blk]
            last = blk == len(blocks) - 1
            yt = yev.tile([128, NB, bw], bf16, tag="yt")
            for cb in range(NB):
                py = ps2.tile([128, bw], f32, tag="py",
                              name=f"py_b{blk}_c{cb}")
                for dt in range(DT):
                    nc.tensor.matmul(
                        py[:],
                        w2s[:, dt, 128 * cb : 128 * (cb + 1)],
                        g[:, dt, t0 : t0 + bw],
                        start=(dt == 0),
                        stop=(dt == DT - 1),
                    )
                if last and cb == NB - 1:
                    # final tile: evict in two column-halves on two rings
                    # so the copy/kick/transfer chains pipeline and only
                    # half a tile trails the last matmul
                    half = bw // 2
                    base = NB * t0 + cb * bw
                    nc.vector.tensor_copy(yt[:, cb, 0:half], py[:, 0:half])
                    nc.sync.dma_start(y[:, base : base + half],
                                      yt[:, cb, 0:half])
                    nc.vector.tensor_copy(yt[:, cb, half:bw], py[:, half:bw])
                    nc.scalar.dma_start(y[:, base + half : base + bw],
                                        yt[:, cb, half:bw])
                    continue
                nc.vector.tensor_copy(yt[:, cb, :], py[:])
                if last:
                    nc.sync.dma_start(
                        y[:, NB * t0 + cb * bw : NB * t0 + (cb + 1) * bw],
                        yt[:, cb, :],
                    )
            if not last:
                nc.sync.dma_start(
                    y[:, NB * t0 : NB * (t0 + bw)].rearrange(
                        "p (cb t) -> p cb t", cb=NB),
                    yt[:],
                )
    return nc


def _route(xf, w_router):
    """Host router: softmax over experts, top-2 (jax tie semantics:
    stable, lower index first), renormalize."""
    logits = xf @ w_router.T                       # [N, E] fp32
    m = logits.max(axis=-1, keepdims=True)
    p = np.exp(logits - m)
    p /= p.sum(axis=-1, keepdims=True)
    topi = np.argsort(-p, axis=-1, kind="stable")[:, :TOP_K]   # [N, 2]
    topw = np.take_along_axis(p, topi, axis=-1)
    topw = topw / topw.sum(axis=-1, keepdims=True)
    return topi.astype(np.int32), topw.astype(np.float32)


def _run_spmd(nc, in_maps, trace=False, trace_cores=None, tmpdir=None):
    from concourse.bass_utils import run_bass_kernel_spmd

    return run_bass_kernel_spmd(
        nc, in_maps, core_ids=list(range(E)),
        trace=trace, trace_cores=trace_cores, tmpdir=tmpdir,
    )


# test.py hooks: set TRACE=True (and optionally TRACE_CORES/TRACE_DIR)
# before calling kernel() to capture an NTFF profile of the run.
TRACE = False
TRACE_CORES = None
TRACE_DIR = None
LAST_RESULT = None


def kernel(x, w_router, w1, b1, w2, b2):
    global LAST_RESULT
    x = np.asarray(x, dtype=np.float32)
    w_router = np.asarray(w_router, dtype=np.float32)
    w1 = np.asarray(w1, dtype=np.float32)
    b1 = np.asarray(b1, dtype=np.float32)
    w2 = np.asarray(w2, dtype=np.float32)
    b2 = np.asarray(b2, dtype=np.float32)

    xf = x.reshape(N, C)
    topi, topw = _route(xf, w_router)

    # token rows routed to each expert (each token appears in exactly 2)
    sel = [np.nonzero((topi == e).any(axis=-1))[0] for e in range(E)]
    max_cnt = max(len(s) for s in sel)
    cap = max(128, -(-max_cnt // 2) * 2)

    if cap not in _cache:
        _cache[cap] = _build(cap)
    nc = _cache[cap]

    blocks = _blocks_of(cap)
    t0s = [sum(blocks[:i]) for i in range(len(blocks))]
    NB = C // 128

    bf16 = ml_dtypes.bfloat16
    xf_bf = xf.astype(bf16)
    in_maps = []
    for e in range(E):
        rows = sel[e]
        xtc = np.zeros((C, cap), dtype=bf16)
        xtc[:, : len(rows)] = xf_bf[rows].T
        # layouts documented in _build; xt packed block-contiguous
        xk = xtc.reshape(C // 128, 128, cap)
        parts = []
        for t0, bw in zip(t0s, blocks):
            parts.append(
                xk[:, :, t0 : t0 + bw].transpose(1, 0, 2).reshape(128, -1))
        xtt = np.ascontiguousarray(np.concatenate(parts, axis=1))
        w1t = w1[e].astype(bf16).reshape(C // 128, 128, D // 512, 512)
        w1t = np.ascontiguousarray(w1t.transpose(2, 1, 0, 3))
        w2t = w2[e].astype(bf16).reshape(D // 512, 4, 128, C)
        w2t = np.ascontiguousarray(w2t.transpose(0, 2, 1, 3))
        in_maps.append({
            "xt": xtt,
            "w1": w1t,
            "w2": w2t,
            "b1t": np.ascontiguousarray(b1[e].reshape(D // 128, 128).T),
        })

    res = _run_spmd(nc, in_maps, trace=TRACE, trace_cores=TRACE_CORES,
                    tmpdir=TRACE_DIR)
    LAST_RESULT = res

    out = np.zeros((N, C), dtype=np.float32)
    for e in range(E):
        rows = sel[e]
        if len(rows) == 0:
            continue
        yp = np.asarray(res.results[e]["y"], dtype=np.float32)  # [128, NB*cap]
        yc = np.empty((C, cap), dtype=np.float32)               # y^T [c, t]
        for t0, bw in zip(t0s, blocks):
            seg = yp[:, NB * t0 : NB * (t0 + bw)].reshape(128, NB, bw)
            yc[:, t0 : t0 + bw] = seg.transpose(1, 0, 2).reshape(C, bw)
        ye = yc.T[: len(rows)]                                  # [n, C]
        # weight of expert e for each selected token
        is_e = topi[rows] == e               # [n_e, 2]
        wgt = (topw[rows] * is_e).sum(axis=-1)
        out[rows] += wgt[:, None] * ye
    # b2 enters after the expert matmul, inside the weighted combine
    out += (topw[:, :, None] * b2[topi]).sum(axis=1)
    return out.reshape(B, T, C)


# revision 57
# speedup vs baseline: 1.0126x; 1.0126x over previous
"""MoE layer (top-2 of 8 experts, exact GELU) on 8 Trainium2 NeuronCores.

Strategy: expert parallelism. The router (0.006% of the FLOPs) runs on the
host; each core holds one expert's weights and runs the two big matmuls
for the tokens routed to that expert:

    H^T = W1[e]^T @ X_e^T          (PE, bf16, accumulated over C in fp32)
    G   = GELU(H^T + b1)           (ACT, fused bias, bf16 out)
    Y   = G^T @ W2[e]              (PE, bf16, accumulated over D in fp32)

Schedule: the kernel is DMA-limited at the start (19 MB of weights +
activations to land) and PE-limited after. Phase 1 runs ALL of matmul1,
d-group-outer, streaming w1 through a 3-deep tile pool -- the PE only
needs xt (2.2 MB) + one 1 MB w1 group to start, and consumes w1 at
~65 GB/s while w2 (8.4 MB) streams in the background. Phase 2 runs all
of matmul2 from the then-fully-resident w2. G for all tokens stays in
SBUF between the phases (freed w1 budget pays for it). This keeps the
PE saturated from ~10us on instead of stalling ~45us for weight groups
(which also dropped the HAM clock gate to half rate).

Computing H transposed (d on partitions) makes the first matmul's output
directly usable as the second matmul's stationary operand -- no on-device
transposes anywhere.
"""

import numpy as np
import ml_dtypes

B, T, C, D, E = 2, 2048, 1024, 4096, 8
N = B * T
TOP_K = 2
NT_BLOCK = 384          # token block width (psum tile free dim)
WARMUP = 48             # PE warm-up matmuls (HAM clock ramp + DMA wait)

_cache = {}


MAX_WAITS = 1  # this walrus build rejects >1 sync wait per instruction


def _install_tile_patch():
    """This container's walrus build rejects >MAX_WAITS sync waits on one
    instruction ("Too many sync wait commands"). Two fixes:
    1. The stock TileContext tail puts every outstanding proc-clock wait on
       a single Drain -- split across sync-engine NOPs, one wait each.
    2. Body instructions can come out of sem assignment with 3+ waits --
       peel the excess onto same-engine NOPs inserted just before."""
    import bass_rust
    import concourse.mybir as mybir
    from concourse import tile
    from concourse.vector_clock import ScopedClock

    if getattr(tile.TileContext, "_drain_patched", False):
        return

    def _patched(self, tick_clock, wait_clock):
        nc = self.nc
        ticks = list(tick_clock.global_clock)
        # round-robin the tail waits across engines: serial on one
        # engine they cost ~1.4us before the final barrier
        engs = [nc.sync, nc.scalar, nc.vector, nc.gpsimd, nc.tensor]
        n = 0
        for p, t in enumerate(ticks):
            if t <= 0:
                continue
            vc = bass_rust.VectorClock()
            vc.require_at_least(p, t)
            nop = engs[n % len(engs)].nop(nofuse=True, hint=f"tail_wait_p{p}")
            n += 1
            wait_clock.add_sem_waits(nop.ins, ScopedClock({None: vc}))
        nc.sync.drain()
        nc.all_engine_barrier()
        assert self.sems is not None
        popped = nc._tile_sem_poison_stack.pop()
        assert popped is self._sem_poison
        nc.clear_and_free_semaphores(list(self.sems.allocated().values()))
        nc.all_engine_barrier()

    tile.TileContext._drain_and_barrier = _patched

    orig_lower = tile.TileContext._lower_ordered_insts

    def _split_excess_waits(self, ordered):
        nc = self.nc
        for bb_name, insts in ordered.items():
            out = []
            for inst in insts:
                si = getattr(inst, "sync_info", None)
                if (
                    si is not None
                    and si.on_wait
                    and len(si.on_wait) > MAX_WAITS
                    and isinstance(inst, mybir.Instruction)
                    and inst.engine != mybir.EngineType.Unassigned
                ):
                    waits = list(si.on_wait)
                    excess, keep = waits[:-MAX_WAITS], waits[-MAX_WAITS:]
                    while excess:
                        chunk, excess = excess[:MAX_WAITS], excess[MAX_WAITS:]
                        nop = mybir.InstNoOp(
                            name=nc.get_next_instruction_name(),
                            sync_info=mybir.SyncInfo(on_wait=chunk, on_update=[]),
                            bass_nofuse=True,
                            engine=inst.engine,
                        )
                        nc.register_instruction(nop)
                        out.append(nop)
                    inst.sync_info = mybir.SyncInfo(
                        on_wait=keep, on_update=list(si.on_update or [])
                    )
                out.append(inst)
            insts[:] = out
        return orig_lower(self, ordered)

    tile.TileContext._lower_ordered_insts = _split_excess_waits
    tile.TileContext._drain_patched = True


def _blocks_of(cap):
    """Token-block widths covering cap. Full NT_BLOCK blocks plus one
    remainder block (kept last so the writeback tail is small)."""
    blocks = [NT_BLOCK] * (cap // NT_BLOCK)
    if cap % NT_BLOCK:
        blocks.append(cap % NT_BLOCK)
    return blocks


def _build(cap):
    """Build the per-core Bass program for token capacity `cap`
    (a multiple of 8)."""
    import concourse.bass as bass
    import concourse.mybir as mybir
    import concourse.tile as tile
    from contextlib import ExitStack

    _install_tile_patch()

    bf16 = mybir.dt.bfloat16
    f32 = mybir.dt.float32
    KC = C // 128            # 8 contraction chunks for matmul1
    DT = D // 128            # 32 d-tiles / contraction chunks for matmul2
    WG = 512                 # w1 d-columns per group (4 d-tiles)
    NG = D // WG             # 8 groups
    NQ = DT // 4             # 8 w2 chunks
    NB = C // 128            # 8 output-channel tiles for matmul2
    GDT = WG // 128          # 4 d-tiles per w1 group
    blocks = _blocks_of(cap)
    t0s = [sum(blocks[:i]) for i in range(len(blocks))]

    nc = bass.Bass()
    # Inputs are host-pre-tiled so every DMA lands with a large contiguous
    # per-partition run (>=6 KB): strided destinations chop transfers into
    # ~1 KB packets that run an HWDGE ring at a fraction of peak.
    #   xt : [128, KC, cap]    xt[p, kc, t]  = x^T[kc*128+p, t]  (block-packed)
    #   w1 : [NG, 128, KC, WG] w1[g, p, kc, j] = w1[kc*128+p, g*WG+j]
    #   w2 : [NQ, 128, 4, C]   w2[q, p, a, c] = w2[(4q+a)*128+p, c]
    # xt is packed block-contiguous: block b at columns [KC*t0, KC*(t0+bw))
    # as (kc, t) so each per-block DMA is one contiguous run per partition.
    xt = nc.declare_dram_parameter("xt", [128, KC * cap], bf16, isOutput=False)
    w1 = nc.declare_dram_parameter("w1", [NG, 128, KC, WG], bf16, isOutput=False)
    w2 = nc.declare_dram_parameter("w2", [NQ, 128, 4, C], bf16, isOutput=False)
    b1t = nc.declare_dram_parameter("b1t", [128, DT], f32, isOutput=False)
    # y^T, block-packed cb-major: block b at columns [NB*t0, NB*(t0+bw))
    # as (cb, t); y[p, NB*t0 + cb*bw + t] = y^T[cb*128+p, t0+t]
    y = nc.declare_dram_parameter("y", [128, NB * cap], bf16, isOutput=True)

    from concourse.bass import _add_dep_helper

    with tile.TileContext(nc) as tc, ExitStack() as ctx:
        const = ctx.enter_context(tc.tile_pool(name="const", bufs=1))
        # per-block xt tiles: each DMA fills a whole tile, so the
        # destination is one contiguous ~6 KB run per partition (a column
        # slice of a shared [128, KC, cap] tile would chop the transfer
        # into 768 B packets and crawl)
        xts = [const.tile([128, KC, bw], bf16, name=f"xts{i}")
               for i, bw in enumerate(blocks)]
        w2s = const.tile([128, DT, C], bf16)
        g = const.tile([128, DT, cap], bf16)
        b1s = const.tile([128, DT], f32)

        w1p = ctx.enter_context(tc.tile_pool(name="w1p", bufs=4))
        ps1 = ctx.enter_context(tc.tile_pool(name="ps1", bufs=4, space="PSUM"))
        ps2 = ctx.enter_context(tc.tile_pool(name="ps2", bufs=3, space="PSUM"))
        yev = ctx.enter_context(tc.tile_pool(name="yev", bufs=2))
        warm = ctx.enter_context(tc.tile_pool(name="warm", bufs=1))

        # DMA plan. Per-ring order == consumption order:
        #   scalar ring: b1, then w1 groups (the matmul1 critical path --
        #       nothing else ever rides this ring, so groups can't be
        #       crowded out; g3..g7 are kicked from inside the loop as
        #       the 3-deep pool recycles).
        #   sync ring: xt blocks (needed within the first w1-group pass),
        #       then half of w2 behind them.
        #   gpsimd SWDGE ring (~13us spin-up): the other half of w2,
        #       which matmul2 only touches after all of matmul1.
        def xt_src(blk):
            t0, bw = t0s[blk], blocks[blk]
            return xt[:, KC * t0 : KC * (t0 + bw)].rearrange(
                "p (kc t) -> p kc t", kc=KC)

        # Ring plan. A single HWDGE ring sustains only ~80-130 GB/s, so
        # the first matmul's critical prefix (w1 g0 + xt b0, ~1.9 MB) is
        # split in kc-halves across rings; the first block's psum groups
        # open on kc 0-3 (needs only the h1 pieces) and close on kc 4-7
        # as the h2 pieces land. Warmup (48 matmuls) bridges the PE at
        # full clock all the way to data-ready (15.5-20us, jittery), so
        # the HAM clock gate never drops:
        #   sync:   xt b0 (2 kc-halves), xt b2, then w2 odd chunks (gated)
        #   scalar: b1, w1 g0 (2 kc-halves), w1 g1-g3 (g4-g7 in-loop)
        #   gpsimd: xt b1, then w2 even chunks (gated)
        w1t = []
        for grp in range(4):
            w1t.append(w1p.tile([128, KC, WG], bf16, tag="w1",
                                name=f"w1t{grp}"))
        nc.sync.dma_start(xts[0][:, 0:4, :], xt_src(0)[:, 0:4, :])
        nc.scalar.dma_start(b1s[:], b1t[:])
        nc.gpsimd.dma_start(xts[1][:], xt_src(1))
        nc.scalar.dma_start(w1t[0][:, 0:4, :], w1[0][:, 0:4, :])
        nc.sync.dma_start(xts[0][:, 4:8, :], xt_src(0)[:, 4:8, :])
        nc.scalar.dma_start(w1t[0][:, 4:8, :], w1[0][:, 4:8, :])
        nc.sync.dma_start(xts[2][:], xt_src(2))
        for grp in range(1, 4):
            nc.scalar.dma_start(w1t[grp][:], w1[grp])
        # w2 is only touched in phase 2 (~125us in); hold its 8.4 MB off
        # the HBM bus until the matmul1 critical path (xt + early w1
        # groups) has landed. A raw dep edge to the w1 DMA only orders
        # the descriptor KICKS, not the transfers, and the Tile scheduler
        # reorders an engine's kicks past a blocked one -- so EVERY chunk
        # gets a real data hazard: a dummy copy that reads a late w1 tile
        # and writes a corner of the w2 destination, which the w2 DMA
        # must wait out (WAW).
        for q in range(NQ):
            eng = nc.gpsimd if q % 2 == 0 else nc.sync
            nc.vector.tensor_copy(w2s[:, 4 * q : 4 * q + 1, 0:1],
                                  w1t[2 + (q % 2)][:, 0:1, 0:1])
            eng.dma_start(w2s[:, 4 * q : 4 * (q + 1), :], w2[q])

        # PE warm-up: the HAM clock gate needs ~3.4us of sustained matmul
        # activity to lift the PE from 1.2 to 2.4 GHz, and the first real
        # matmul can't start until xt block 0 + w1 group 0 land (~12us).
        # Burn the wait on dummy matmuls over zeroed scratch, ping-ponging
        # two psum tiles borrowed from the (idle) ps2 pool -- a single
        # tile WAW-serializes each matmul on the previous one's drain.
        wsrc = warm.tile([128, NT_BLOCK], bf16)
        nc.vector.memset(wsrc[:], 0.0)
        wps = [ps2.tile([128, NT_BLOCK], f32, tag="py", name=f"wps{i}")
               for i in range(2)]
        for i in range(WARMUP):
            nc.tensor.matmul(wps[i % 2][:], wsrc[:, :128], wsrc[:],
                             start=True, stop=True)

        # Phase 1: all of matmul1, d-group-outer so each 1 MB w1 group is
        # fully consumed (4 d-tiles x all token blocks) before the next is
        # needed -- ~15us of PE work per group vs ~4us to stream one in.
        # Block-outer within the group so the very first psum group needs
        # only xt b0 (not all three blocks).
        for grp in range(NG):
            for blk, bw in enumerate(blocks):
                t0 = t0s[blk]
                if grp == 0 and blk == 0:
                    # First block: all four psum groups open on kc 0-3,
                    # close on kc 4-7, so the PE starts as soon as the
                    # h1 DMA pieces land instead of waiting for all of
                    # w1 g0 + xt b0.
                    phs = [ps1.tile([128, bw], f32, tag="ph",
                                    name=f"ph0_{dtl}") for dtl in range(GDT)]
                    for half in range(2):
                        for dtl in range(GDT):
                            for kc in range(4 * half, 4 * half + 4):
                                nc.tensor.matmul(
                                    phs[dtl][:],
                                    w1t[0][:, kc, 128 * dtl : 128 * (dtl + 1)],
                                    xts[0][:, kc, :],
                                    start=(kc == 0),
                                    stop=(kc == KC - 1),
                                )
                    for dtl in range(GDT):
                        nc.scalar.activation(
                            g[:, dtl, t0 : t0 + bw], phs[dtl][:],
                            mybir.ActivationFunctionType.Gelu,
                            bias=b1s[:, dtl : dtl + 1],
                        )
                    continue
                for dtl in range(GDT):
                    dt = GDT * grp + dtl
                    ph = ps1.tile([128, bw], f32, tag="ph")
                    for kc in range(KC):
                        nc.tensor.matmul(
                            ph[:],
                            w1t[grp][:, kc, 128 * dtl : 128 * (dtl + 1)],
                            xts[blk][:, kc, :],
                            start=(kc == 0),
                            stop=(kc == KC - 1),
                        )
                    nc.scalar.activation(
                        g[:, dt, t0 : t0 + bw], ph[:],
                        mybir.ActivationFunctionType.Gelu,
                        bias=b1s[:, dt : dt + 1],
                    )
            if grp + 4 < NG:
                w1t.append(w1p.tile([128, KC, WG], bf16, tag="w1",
                                    name=f"w1t{grp + 4}"))
                nc.scalar.dma_start(w1t[grp + 4][:], w1[grp + 4])

        # Phase 2: all of matmul2 from resident w2, cb-outer per block.
        # y^T: lhsT = w2 tile (stationary), rhs = g (tokens moving). The
        # last (smallest) block writes back per-cb so only a sliver of
        # DMA trails the final matmul.
        for blk, bw in enumerate(blocks):
            t0 = t0s[blk]
            last = blk == len(blocks) - 1
            yt = yev.tile([128, NB, bw], bf16, tag="yt")
            for cb in range(NB):
                py = ps2.tile([128, bw], f32, tag="py",
                              name=f"py_b{blk}_c{cb}")
                for dt in range(DT):
                    nc.tensor.matmul(
                        py[:],
                        w2s[:, dt, 128 * cb : 128 * (cb + 1)],
                        g[:, dt, t0 : t0 + bw],
                        start=(dt == 0),
                        stop=(dt == DT - 1),
                    )
                if last and cb == NB - 1:
                    # final tile: evict in two column-halves on two rings
                    # so the copy/kick/transfer chains pipeline and only
                    # half a tile trails the last matmul
                    half = bw // 2
                    base = NB * t0 + cb * bw
                    nc.vector.tensor_copy(yt[:, cb, 0:half], py[:, 0:half])
                    nc.sync.dma_start(y[:, base : base + half],
                                      yt[:, cb, 0:half])
                    nc.vector.tensor_copy(yt[:, cb, half:bw], py[:, half:bw])
                    nc.scalar.dma_start(y[:, base + half : base + bw],
                                        yt[:, cb, half:bw])
                    continue
                nc.vector.tensor_copy(yt[:, cb, :], py[:])
                if last:
                    nc.sync.dma_start(
                        y[:, NB * t0 + cb * bw : NB * t0 + (cb + 1) * bw],
                        yt[:, cb, :],
                    )
            if not last:
                nc.sync.dma_start(
                    y[:, NB * t0 : NB * (t0 + bw)].rearrange(
                        "p (cb t) -> p cb t", cb=NB),
                    yt[:],
                )
    return nc


def _route(xf, w_router):
    """Host router: softmax over experts, top-2 (jax tie semantics:
    stable, lower index first), renormalize."""
    logits = xf @ w_router.T                       # [N, E] fp32
    m = logits.max(axis=-1, keepdims=True)
    p = np.exp(logits - m)
    p /= p.sum(axis=-1, keepdims=True)
    topi = np.argsort(-p, axis=-1, kind="stable")[:, :TOP_K]   # [N, 2]
    topw = np.take_along_axis(p, topi, axis=-1)
    topw = topw / topw.sum(axis=-1, keepdims=True)
    return topi.astype(np.int32), topw.astype(np.float32)


def _run_spmd(nc, in_maps, trace=False, trace_cores=None, tmpdir=None):
    from concourse.bass_utils import run_bass_kernel_spmd

    return run_bass_kernel_spmd(
        nc, in_maps, core_ids=list(range(E)),
        trace=trace, trace_cores=trace_cores, tmpdir=tmpdir,
    )


# test.py hooks: set TRACE=True (and optionally TRACE_CORES/TRACE_DIR)
# before calling kernel() to capture an NTFF profile of the run.
TRACE = False
TRACE_CORES = None
TRACE_DIR = None
LAST_RESULT = None


def kernel(x, w_router, w1, b1, w2, b2):
    global LAST_RESULT
    x = np.asarray(x, dtype=np.float32)
    w_router = np.asarray(w_router, dtype=np.float32)
    w1 = np.asarray(w1, dtype=np.float32)
    b1 = np.asarray(b1, dtype=np.float32)
    w2 = np.asarray(w2, dtype=np.float32)
    b2 = np.asarray(b2, dtype=np.float32)

    xf = x.reshape(N, C)
    topi, topw = _route(xf, w_router)

    # token rows routed to each expert (each token appears in exactly 2)
    sel = [np.nonzero((topi == e).any(axis=-1))[0] for e in range(E)]
    max_cnt = max(len(s) for s in sel)
    cap = max(128, -(-max_cnt // 2) * 2)

    if cap not in _cache:
        _cache[cap] = _build(cap)
    nc = _cache[cap]

    blocks = _blocks_of(cap)
    t0s = [sum(blocks[:i]) for i in range(len(blocks))]
    NB = C // 128

    bf16 = ml_dtypes.bfloat16
    xf_bf = xf.astype(bf16)
    in_maps = []
    for e in range(E):
        rows = sel[e]
        xtc = np.zeros((C, cap), dtype=bf16)
        xtc[:, : len(rows)] = xf_bf[rows].T
        # layouts documented in _build; xt packed block-contiguous
        xk = xtc.reshape(C // 128, 128, cap)
        parts = []
        for t0, bw in zip(t0s, blocks):
            parts.append(
                xk[:, :, t0 : t0 + bw].transpose(1, 0, 2).reshape(128, -1))
        xtt = np.ascontiguousarray(np.concatenate(parts, axis=1))
        w1t = w1[e].astype(bf16).reshape(C // 128, 128, D // 512, 512)
        w1t = np.ascontiguousarray(w1t.transpose(2, 1, 0, 3))
        w2t = w2[e].astype(bf16).reshape(D // 512, 4, 128, C)
        w2t = np.ascontiguousarray(w2t.transpose(0, 2, 1, 3))
        in_maps.append({
            "xt": xtt,
            "w1": w1t,
            "w2": w2t,
            "b1t": np.ascontiguousarray(b1[e].reshape(D // 128, 128).T),
        })

    res = _run_spmd(nc, in_maps, trace=TRACE, trace_cores=TRACE_CORES,
                    tmpdir=TRACE_DIR)
    LAST_RESULT = res

    out = np.zeros((N, C), dtype=np.float32)
    for e in range(E):
        rows = sel[e]
        if len(rows) == 0:
            continue
        yp = np.asarray(res.results[e]["y"], dtype=np.float32)  # [128, NB*cap]
        yc = np.empty((C, cap), dtype=np.float32)               # y^T [c, t]
        for t0, bw in zip(t0s, blocks):
            seg = yp[:, NB * t0 : NB * (t0 + bw)].reshape(128, NB, bw)
            yc[:, t0 : t0 + bw] = seg.transpose(1, 0, 2).reshape(C, bw)
        ye = yc.T[: len(rows)]                                  # [n, C]
        # weight of expert e for each selected token
        is_e = topi[rows] == e               # [n_e, 2]
        wgt = (topw[rows] * is_e).sum(axis=-1)
        out[rows] += wgt[:, None] * ye
    # b2 enters after the expert matmul, inside the weighted combine
    out += (topw[:, :, None] * b2[topi]).sum(axis=1)
    return out.reshape(B, T, C)
